# revision 1
# baseline (speedup 1.0000x reference)
"""Cross-covariance-style attention (XCA variant, no q/k transpose) on 8 TRN2 cores.

Reference computation (per batch element b, H=8 heads, hd=96):
    qkv = x @ w_qkv                      # [N=1024, 3C], C=768
    q, k, v = split(qkv)                 # each [H, N, hd] logically
    qn = q / ||q||_row;  kn = k / ||k||_row
    S = (qn @ kn^T) * temperature        # [H, N, N]
    P = softmax(S, axis=-1)
    out = P @ v                          # [H, N, hd]
    y = out @ w_proj + b_proj            # [N, C]

Sharding: data-parallel over batch B=8 -> one batch element per NeuronCore,
no collectives.  Each core runs the identical program on its slice.

Per-core dataflow (v2 — everything bf16 except stats/denominators):
  - xT loaded via 6 DMA-transposes (bf16), w_qkv loaded bf16.
  - q^T and k^T are produced DIRECTLY in transposed layout [d=96, n] by the
    projection matmul with swapped operands (lhsT = w_qkv head-column slice,
    rhs = xT).  No per-head transposes at all.
  - Row sum-of-squares: ACT squares each projection PSUM chunk, then an
    indicator-matrix matmul (lhsT column j = ones for slice j) accumulates
    all 16 q/k head norms into a single [16, 512] PSUM tile — lane-parallel,
    so sqrt (ACT) and reciprocal (DVE) run once on [16, 1024].
  - q^T scaled by 1/||q|| (row-broadcast via a tiny DRAM bounce), temperature
    / ||k|| transposed into per-m-chunk [128, 8] tiles for the Exp scale.
  - S^T = k^T.T @ qn^T per head -> Exp on ACT straight from PSUM -> P^T bf16.
  - out^T = [v | 1]-extended PV matmul: PSUM row 96 accumulates the softmax
    denominator for free; out^T is normalized with a DVE divide against the
    DRAM-bounce-broadcast denominator row.
  - v is copied from the natural-orientation projection into [v | 1] tiles.
  - Projection: lhsT = out^T per head (K=96 accumulation), bf16, plus bias.
"""

import os

import numpy as np
import ml_dtypes

import concourse.bass as bass
import concourse.tile as tile
import concourse.mybir as mybir
from concourse.vector_clock import ScopedClock
from concourse.bass_utils import run_bass_kernel_spmd

B, N, C = 8, 1024, 768
H, HD = 8, 96
NM = N // 128          # 8 row chunks of 128
KC = C // 128          # 6 contraction chunks
NB = 384               # v-projection output column chunk
F32 = mybir.dt.float32
BF16 = mybir.dt.bfloat16
AF = mybir.ActivationFunctionType


class SafeTileContext(tile.TileContext):
    """This toolchain's walrus rejects >1 sync wait per instruction and the
    EVENT_SEMAPHORE_RANGE_CLEAR ISA op; patch the end-of-context quiesce."""

    MAXW = 1

    def _drain_and_barrier(self, tick_clock, wait_clock):
        nc = self.nc
        drain_inst = nc.sync.drain()
        wait_clock.add_sem_waits(
            drain_inst.ins, ScopedClock({None: tick_clock.global_clock})
        )
        si = drain_inst.ins.sync_info
        waits = list(si.on_wait or [])
        if len(waits) > self.MAXW:
            si.on_wait = waits[: self.MAXW]
            rest = waits[self.MAXW :]
            for i in range(0, len(rest), self.MAXW):
                nop = nc.sync.nop()
                nsi = nop.ins.sync_info
                chunk = rest[i : i + self.MAXW]
                if nsi is None:
                    nop.ins.sync_info = mybir.SyncInfo(on_wait=chunk, on_update=[])
                else:
                    nsi.on_wait = list(nsi.on_wait or []) + chunk
                    nop.ins.sync_info = nsi
        nc.all_engine_barrier()
        popped = nc._tile_sem_poison_stack.pop()
        assert popped is self._sem_poison
        sems = list(self.sems.allocated().values())
        if sems:
            sem_nums = [s.num if hasattr(s, "num") else int(s) for s in sems]
            for i, num in enumerate(sem_nums):
                inst = mybir.InstEventSemaphore(
                    name=f"semwr-{num}-{i}", ins=[], outs=[]
                )
                inst.engine = mybir.EngineType.Pool
                inst.sync_info = mybir.SyncInfo(
                    on_wait=[],
                    on_update=[
                        mybir.SyncUpdate(
                            id=num, sync_type="semaphore",
                            update_mode="sem-wr-imm", update_value=0,
                        )
                    ],
                )
                nc.register_instruction(inst)
                nc.cur_bb.bb.add_instruction(inst)
            nc._state.prepend_free_semaphores(sem_nums)
            for poison_set in nc._tile_sem_poison_stack:
                poison_set.update(sem_nums)
        nc.all_engine_barrier()


def _split_multi_waits(nc):
    """This walrus encodes at most ONE sync wait per instruction.  Hoist
    extra waits onto same-engine InstNoOp's placed just before the offending
    instruction (engines execute their stream in order)."""
    counter = 0
    for f in nc.m.functions:
        for bb in f.blocks:
            insts = list(bb.instructions)
            out = []
            changed = False
            for inst in insts:
                si = inst.sync_info
                waits = list(si.on_wait) if si and si.on_wait else []
                if len(waits) > 1 and inst.engine != mybir.EngineType.Unassigned:
                    for w in waits[:-1]:
                        counter += 1
                        nop = mybir.InstNoOp(name=f"swsplit-{counter}", ins=[], outs=[])
                        nop.engine = inst.engine
                        nop.sync_info = mybir.SyncInfo(on_wait=[w], on_update=[])
                        nc.register_instruction(nop)
                        out.append(nop)
                    si.on_wait = [waits[-1]]
                    inst.sync_info = si
                    changed = True
                out.append(inst)
            if changed:
                bb.instructions = out
    return nc


def _bcast_ap(ap, parts):
    """DRAM AP replicated across `parts` partitions (step-0 leading dim)."""
    return bass.AP(tensor=ap.tensor, offset=ap.offset,
                   ap=[[0, parts]] + list(ap.ap)[-1:])


def build():
    nc = bass.Bass("TRN2")
    x = nc.dram_tensor("x", [N, C], BF16, kind="ExternalInput")
    w_qkv = nc.dram_tensor("w_qkv", [C, 3 * C], BF16, kind="ExternalInput")
    temp = nc.dram_tensor("temperature", [H], F32, kind="ExternalInput")
    w_proj = nc.dram_tensor("w_proj", [C, C], BF16, kind="ExternalInput")
    b_proj = nc.dram_tensor("b_proj", [C], F32, kind="ExternalInput")
    y = nc.dram_tensor("y", [N, C], F32, kind="ExternalOutput")

    wq_t = w_qkv.rearrange("(k p) n -> k p n", p=128)   # [6, 128, 2304]
    wp_t = w_proj.rearrange("(h d) j -> h d j", d=HD)   # [8, 96, 768]

    with SafeTileContext(nc) as tc:
        with tc.tile_pool(name="persist", bufs=1) as pp, \
             tc.tile_pool(name="small", bufs=1) as sp, \
             tc.tile_pool(name="dram", bufs=1, space="DRAM") as dp:
            # ---- constants / weights ----
            b_bcast = sp.tile([128, C], F32, name="b_bcast")
            nc.gpsimd.dma_start(out=b_bcast, in_=_bcast_ap(b_proj[:], 128))
            # temp_col rows 0..7 = 1.0 (q), rows 8..15 = temperature (k)
            temp_col = sp.tile([16, 1], F32, name="temp_col")
            nc.vector.memset(temp_col[0:8, :], 1.0)
            nc.sync.dma_start(out=temp_col[8:16, :], in_=temp[:])
            wproj_sb = []
            for h in range(H):
                t = pp.tile([HD, C], BF16, name=f"wp{h}")
                nc.scalar.dma_start(out=t, in_=wp_t[h])
                wproj_sb.append(t)
            # indicator pack: Epack[:, t*16 + t] = 1, else 0
            Epack = sp.tile([HD, 16, 16], BF16, name="Epack")
            nc.vector.memset(Epack, 0.0)
            nc.vector.memset(
                bass.AP(tensor=Epack.tensor, offset=Epack.offset,
                        ap=list(Epack.ap)[:1] + [[17, 16]]),
                1.0,
            )

            # ---- persistent activation tensors ----
            qT = [pp.tile([HD, N], BF16, name=f"qT{h}") for h in range(H)]
            kT = [pp.tile([HD, N], BF16, name=f"kT{h}") for h in range(H)]
            vext = [pp.tile([128, H, HD + 1], BF16, name=f"v{m}") for m in range(NM)]
            rkt_t = [sp.tile([128, H], F32, name=f"rkt{m}") for m in range(NM)]
            ss_sb = sp.tile([16, N], F32, name="ss")
            outT = [pp.tile([HD, N], BF16, name=f"oT{h}") for h in range(H)]
            for m in range(NM):
                nc.vector.memset(vext[m], 1.0)

            # ================= phase 1: projections + norms =================
            with tc.tile_pool(name="wqkv", bufs=1) as wqp, \
                 tc.tile_pool(name="xT", bufs=1) as xtp, \
                 tc.tile_pool(name="p1_ps", bufs=4, space="PSUM") as qkp, \
                 tc.tile_pool(name="ss_ps", bufs=1, space="PSUM") as ssp, \
                 tc.tile_pool(name="sq", bufs=3) as sqp:
                wqkv_sb = []
                for kk in range(KC):
                    t = wqp.tile([128, 3 * C], BF16, name=f"wq{kk}")
                    nc.sync.dma_start(out=t, in_=wq_t[kk])
                    wqkv_sb.append(t)
                xT_sb = []
                for kk in range(KC):
                    t = xtp.tile([128, N], BF16, name=f"xT{kk}")
                    nc.sync.dma_start(
                        out=t, in_=x[:, kk * 128 : (kk + 1) * 128], transpose=True
                    )
                    xT_sb.append(t)

                # -- q^T / k^T directly transposed, + stacked sum-of-squares --
                ss_ps = [ssp.tile([16, 512], F32, name=f"ssp{nc2}") for nc2 in range(2)]
                for t_i in range(16):  # 0..7 q-heads, 8..15 k-heads
                    col0 = t_i * HD if t_i < 8 else C + (t_i - 8) * HD
                    dst = qT[t_i] if t_i < 8 else kT[t_i - 8]
                    for nc2 in range(2):
                        ps = qkp.tile([HD, 512], F32, name="qk")
                        for kk in range(KC):
                            nc.tensor.matmul(
                                ps,
                                lhsT=wqkv_sb[kk][:, col0 : col0 + HD],
                                rhs=xT_sb[kk][:, nc2 * 512 : (nc2 + 1) * 512],
                                start=(kk == 0),
                                stop=(kk == KC - 1),
                            )
                        sq = sqp.tile([HD, 512], BF16, name="sq")
                        nc.scalar.activation(out=sq, in_=ps, func=AF.Square)
                        nc.vector.tensor_copy(
                            out=dst[:, nc2 * 512 : (nc2 + 1) * 512], in_=ps
                        )
                        nc.tensor.matmul(
                            ss_ps[nc2],
                            lhsT=Epack[:, t_i, :],
                            rhs=sq,
                            start=(t_i == 0),
                            stop=(t_i == 15),
                        )
                # rsq = temp_col / sqrt(ss)  (rows 0..7 rq, 8..15 rk*temp)
                for nc2 in range(2):
                    nc.vector.tensor_copy(
                        out=ss_sb[:, nc2 * 512 : (nc2 + 1) * 512], in_=ss_ps[nc2]
                    )
                nc.scalar.activation(out=ss_sb, in_=ss_sb, func=AF.Sqrt)
                nc.vector.reciprocal(out=ss_sb, in_=ss_sb)
                nc.vector.tensor_scalar_mul(out=ss_sb, in0=ss_sb, scalar1=temp_col)

                # rk*temp rows -> per-m-chunk [128, 8] via a DRAM bounce
                # (DMA can scatter across partitions; engines cannot)
                rk_d = dp.tile([H, N], F32, name="rk_d")
                nc.sync.dma_start(out=rk_d, in_=ss_sb[8:16, :])
                for m in range(NM):
                    nc.gpsimd.dma_start(
                        out=rkt_t[m],
                        in_=bass.AP(
                            tensor=rk_d.tensor,
                            offset=rk_d.offset + m * 128,
                            ap=[[1, 128], [N, H]],
                        ),
                    )
                # scale q^T rows by rq (row-broadcast via DRAM bounce)
                for h in range(H):
                    dscr = dp.tile([1, N], F32, name=f"rqd{h}")
                    nc.scalar.dma_start(out=dscr, in_=ss_sb[h : h + 1, :])
                    rqb = sqp.tile([HD, N], F32, name="rqb")
                    nc.gpsimd.dma_start(out=rqb, in_=_bcast_ap(dscr, HD))
                    nc.vector.tensor_mul(out=qT[h], in0=qT[h], in1=rqb)

                # -- v in natural orientation into [v | 1] tiles --
                for nb in range(2):
                    for m in range(NM):
                        ps = qkp.tile([128, NB], F32, name="qk")
                        for kk in range(KC):
                            nc.tensor.matmul(
                                ps,
                                lhsT=xT_sb[kk][:, m * 128 : (m + 1) * 128],
                                rhs=wqkv_sb[kk][
                                    :, 2 * C + nb * NB : 2 * C + (nb + 1) * NB
                                ],
                                start=(kk == 0),
                                stop=(kk == KC - 1),
                            )
                        nc.vector.tensor_copy(
                            out=vext[m][:, nb * 4 : (nb + 1) * 4, :HD],
                            in_=ps.rearrange("p (hh d) -> p hh d", d=HD),
                        )

            # ================= phase 2: attention per head =================
            with tc.tile_pool(name="pT", bufs=2) as ptp, \
                 tc.tile_pool(name="dn", bufs=3) as dnp, \
                 tc.tile_pool(name="s_ps", bufs=4, space="PSUM") as spp, \
                 tc.tile_pool(name="o_ps", bufs=3, space="PSUM") as opp, \
                 tc.tile_pool(name="dnd", bufs=3, space="DRAM") as ddp:
                for h in range(H):
                    pTs = []
                    for m in range(NM):
                        pTm = ptp.tile([128, N], BF16, name=f"pT{m}")
                        for nc2 in range(2):
                            ps = spp.tile([128, 512], F32, name="s")
                            nc.tensor.matmul(
                                ps,
                                lhsT=kT[h][:, m * 128 : (m + 1) * 128],
                                rhs=qT[h][:, nc2 * 512 : (nc2 + 1) * 512],
                                start=True, stop=True,
                            )
                            nc.scalar.activation(
                                out=pTm[:, nc2 * 512 : (nc2 + 1) * 512],
                                in_=ps,
                                func=AF.Exp,
                                scale=rkt_t[m][:, h : h + 1],
                            )
                        pTs.append(pTm)
                    for nc2 in range(2):
                        po = opp.tile([HD + 1, 512], F32, name="po")
                        for m in range(NM):
                            nc.tensor.matmul(
                                po,
                                lhsT=vext[m][:, h, :],
                                rhs=pTs[m][:, nc2 * 512 : (nc2 + 1) * 512],
                                start=(m == 0),
                                stop=(m == NM - 1),
                            )
                        den = dnp.tile([HD + 1, 512], F32, name="den")
                        nc.vector.tensor_copy(
                            out=den[HD : HD + 1, :], in_=po[HD : HD + 1, :]
                        )
                        nc.vector.reciprocal(
                            out=den[HD : HD + 1, :], in_=den[HD : HD + 1, :]
                        )
                        dscr = ddp.tile([1, 512], F32, name="dscr")
                        nc.scalar.dma_start(out=dscr, in_=den[HD : HD + 1, :])
                        rb = dnp.tile([HD, 512], F32, name="rb")
                        nc.gpsimd.dma_start(out=rb, in_=_bcast_ap(dscr, HD))
                        nc.vector.tensor_mul(
                            out=outT[h][:, nc2 * 512 : (nc2 + 1) * 512],
                            in0=po[:HD, :],
                            in1=rb,
                        )

            # ================= phase 3: projection + bias =================
            with tc.tile_pool(name="y_ps", bufs=4, space="PSUM") as ypp, \
                 tc.tile_pool(name="ysb", bufs=2) as ysp:
                for m in range(NM):
                    ym = ysp.tile([128, C], F32, name="ym")
                    for jb in range(2):
                        py = ypp.tile([128, NB], F32, name="py")
                        for h in range(H):
                            nc.tensor.matmul(
                                py,
                                lhsT=outT[h][:, m * 128 : (m + 1) * 128],
                                rhs=wproj_sb[h][:, jb * NB : (jb + 1) * NB],
                                start=(h == 0),
                                stop=(h == H - 1),
                            )
                        nc.vector.tensor_add(
                            out=ym[:, jb * NB : (jb + 1) * NB],
                            in0=py,
                            in1=b_bcast[:, jb * NB : (jb + 1) * NB],
                        )
                    nc.scalar.dma_start(
                        out=y[m * 128 : (m + 1) * 128, :], in_=ym
                    )
    return _split_multi_waits(nc)


_NC = None
LAST_RESULT = None


def kernel(x, w_qkv, temperature, w_proj, b_proj):
    global _NC, LAST_RESULT
    if _NC is None:
        _NC = build()
    xb = np.asarray(x, dtype=np.float32).astype(ml_dtypes.bfloat16)
    wqb = np.asarray(w_qkv, dtype=np.float32).astype(ml_dtypes.bfloat16)
    tf = np.ascontiguousarray(np.asarray(temperature, dtype=np.float32).reshape(H))
    wp = np.asarray(w_proj, dtype=np.float32).astype(ml_dtypes.bfloat16)
    bp = np.ascontiguousarray(np.asarray(b_proj, dtype=np.float32))
    in_maps = [
        {
            "x": np.ascontiguousarray(xb[i]),
            "w_qkv": np.ascontiguousarray(wqb),
            "temperature": tf,
            "w_proj": wp,
            "b_proj": bp,
        }
        for i in range(B)
    ]
    trace = bool(int(os.environ.get("KERNEL_TRACE", "0")))
    res = run_bass_kernel_spmd(
        _NC, in_maps, core_ids=list(range(B)), trace=trace
    )
    LAST_RESULT = res
    out = np.stack([res.results[i]["y"] for i in range(B)], axis=0)
    return out.astype(np.float32)



# revision 18
# speedup vs baseline: 1.0558x; 1.0558x over previous
"""Cross-covariance-style attention (XCA variant, no q/k transpose) on 8 TRN2 cores.

Reference computation (per batch element b, H=8 heads, hd=96):
    qkv = x @ w_qkv                      # [N=1024, 3C], C=768
    q, k, v = split(qkv)                 # each [H, N, hd] logically
    qn = q / ||q||_row;  kn = k / ||k||_row
    S = (qn @ kn^T) * temperature        # [H, N, N]
    P = softmax(S, axis=-1)
    out = P @ v                          # [H, N, hd]
    y = out @ w_proj + b_proj            # [N, C]

Sharding: data-parallel over batch B=8 -> one batch element per NeuronCore,
no collectives.

v3 dataflow (all bf16 matmuls; stats in f32):
  - x^T is transposed on the HOST and fed as `xt` [C, N] bf16 -> plain
    contiguous DMA loads, no DMA transposes.
  - q^T/k^T produced directly transposed by swapped-operand projection,
    accumulation split into two 3-chunk PSUM groups (A: kk 0-2, B: kk 3-5)
    summed by the DVE PSUM->SBUF move, so matmuls start once half the
    weights have landed.  k-heads first, q-heads second, v last, so the
    norm chains overlap later projections.
  - Row sum-of-squares via indicator matmuls into per-side [8, 512] PSUM
    tiles; sqrt (ACT) + reciprocal_approx_fast (custom DVE).
  - temp/||k|| rows -> per-m [128, 8] EXP-scale tiles via PE transpose.
  - 1/||q|| broadcast across partitions via a selector matmul
    (sel[:, h, :] [8, 96] x rq_bf [8, 512] -> [96, 512] PSUM), then DVE
    multiply normalizes q^T.
  - Phase 2 per head: S^T into a 2-bank [128, 1024] PSUM tile (2 matmuls),
    ONE wide Exp -> P^T bf16 with per-partition scale temp/||k||;
    PV via [v | 1] into wide [97, 1024] PSUM (row 96 = softmax denom);
    denom: reciprocal_approx_fast on partition 96, ones-matmul broadcast
    (lhsT/rhs based at partition 96, tile_position (96, 0)) -> DVE mul.
  - Phase 3: per m-chunk [128, 768] 2-bank PSUM, 8-head accumulation,
    DVE bias add, DMA out.
"""

import os

import numpy as np
import ml_dtypes

import concourse.bass as bass
import concourse.tile as tile
import concourse.mybir as mybir
from concourse.vector_clock import ScopedClock
from concourse.bass_utils import run_bass_kernel_spmd

B, N, C = 8, 1024, 768
H, HD = 8, 96
NM = N // 128          # 8 row chunks of 128
KC = C // 128          # 6 contraction chunks
F32 = mybir.dt.float32
BF16 = mybir.dt.bfloat16
AF = mybir.ActivationFunctionType


class SafeTileContext(tile.TileContext):
    """This toolchain's walrus rejects >1 sync wait per instruction and the
    EVENT_SEMAPHORE_RANGE_CLEAR ISA op; patch the end-of-context quiesce."""

    MAXW = 1

    def _drain_and_barrier(self, tick_clock, wait_clock):
        nc = self.nc
        drain_inst = nc.sync.drain()
        wait_clock.add_sem_waits(
            drain_inst.ins, ScopedClock({None: tick_clock.global_clock})
        )
        si = drain_inst.ins.sync_info
        waits = list(si.on_wait or [])
        if len(waits) > self.MAXW:
            si.on_wait = waits[: self.MAXW]
            rest = waits[self.MAXW :]
            for i in range(0, len(rest), self.MAXW):
                nop = nc.sync.nop()
                nsi = nop.ins.sync_info
                chunk = rest[i : i + self.MAXW]
                if nsi is None:
                    nop.ins.sync_info = mybir.SyncInfo(on_wait=chunk, on_update=[])
                else:
                    nsi.on_wait = list(nsi.on_wait or []) + chunk
                    nop.ins.sync_info = nsi
        nc.all_engine_barrier()
        popped = nc._tile_sem_poison_stack.pop()
        assert popped is self._sem_poison
        sems = list(self.sems.allocated().values())
        if sems:
            sem_nums = [s.num if hasattr(s, "num") else int(s) for s in sems]
            for i, num in enumerate(sem_nums):
                inst = mybir.InstEventSemaphore(
                    name=f"semwr-{num}-{i}", ins=[], outs=[]
                )
                inst.engine = mybir.EngineType.Pool
                inst.sync_info = mybir.SyncInfo(
                    on_wait=[],
                    on_update=[
                        mybir.SyncUpdate(
                            id=num, sync_type="semaphore",
                            update_mode="sem-wr-imm", update_value=0,
                        )
                    ],
                )
                nc.register_instruction(inst)
                nc.cur_bb.bb.add_instruction(inst)
            nc._state.prepend_free_semaphores(sem_nums)
            for poison_set in nc._tile_sem_poison_stack:
                poison_set.update(sem_nums)
        nc.all_engine_barrier()


def _split_multi_waits(nc):
    """This walrus encodes at most ONE sync wait per instruction.  Hoist
    extra waits onto same-engine InstNoOp's placed just before the offending
    instruction (engines execute their stream in order)."""
    counter = 0
    for f in nc.m.functions:
        for bb in f.blocks:
            insts = list(bb.instructions)
            out = []
            changed = False
            for inst in insts:
                si = inst.sync_info
                waits = list(si.on_wait) if si and si.on_wait else []
                if len(waits) > 1 and inst.engine != mybir.EngineType.Unassigned:
                    for w in waits[:-1]:
                        counter += 1
                        nop = mybir.InstNoOp(name=f"swsplit-{counter}", ins=[], outs=[])
                        nop.engine = inst.engine
                        nop.sync_info = mybir.SyncInfo(on_wait=[w], on_update=[])
                        nc.register_instruction(nop)
                        out.append(nop)
                    si.on_wait = [waits[-1]]
                    inst.sync_info = si
                    changed = True
                out.append(inst)
            if changed:
                bb.instructions = out
    return nc


def _bcast_ap(ap, parts):
    """DRAM AP replicated across `parts` partitions (step-0 leading dim)."""
    return bass.AP(tensor=ap.tensor, offset=ap.offset,
                   ap=[[0, parts]] + list(ap.ap)[-1:])


def build():
    nc = bass.Bass("TRN2")
    xt = nc.dram_tensor("xt", [C, N], BF16, kind="ExternalInput")
    w_qkv = nc.dram_tensor("w_qkv", [C, 3 * C], BF16, kind="ExternalInput")
    temp = nc.dram_tensor("temperature", [H], F32, kind="ExternalInput")
    w_proj = nc.dram_tensor("w_proj", [C, C], BF16, kind="ExternalInput")
    b_proj = nc.dram_tensor("b_proj", [C], F32, kind="ExternalInput")
    y = nc.dram_tensor("y", [N, C], F32, kind="ExternalOutput")

    xt_t = xt.rearrange("(k p) n -> k p n", p=128)      # [6, 128, 1024]
    wq_t = w_qkv.rearrange("(k p) n -> k p n", p=128)   # [6, 128, 2304]
    wp_t = w_proj.rearrange("(h d) j -> h d j", d=HD)   # [8, 96, 768]

    with SafeTileContext(nc) as tc:
        with tc.tile_pool(name="persist", bufs=1) as pp, \
             tc.tile_pool(name="small", bufs=1) as sp:
            # ---- weights / constants (issue loads first, interleaved) ----
            wqkv_sb = [pp.tile([128, 3 * C], BF16, name=f"wq{kk}") for kk in range(KC)]
            xt_sb = [pp.tile([128, N], BF16, name=f"xT{kk}") for kk in range(KC)]
            for kk in range(KC):
                nc.sync.dma_start(out=wqkv_sb[kk], in_=wq_t[kk])
                nc.gpsimd.dma_start(out=xt_sb[kk], in_=xt_t[kk])
            wproj_sb = []
            for h in range(H):
                t = pp.tile([HD, C], BF16, name=f"wp{h}")
                nc.scalar.dma_start(out=t, in_=wp_t[h])
                wproj_sb.append(t)
            b_bcast = sp.tile([128, C], F32, name="b_bcast")
            nc.scalar.dma_start(out=b_bcast, in_=_bcast_ap(b_proj[:], 128))
            # temperature as [8,1] column (per-partition scalar for k rows)
            temp_col = sp.tile([H, 1], F32, name="temp_col")
            nc.scalar.dma_start(out=temp_col, in_=temp[:])

            # host-prepared constants: indicator pack Es[:, t, j] = (j == t),
            # rq-broadcast selector sel[k, c, j] = (k == j), 8x8 identity
            es_d = nc.dram_tensor("es_c", [HD, 8 * 8], BF16, kind="ExternalInput")
            sel_d = nc.dram_tensor("sel_c", [H, HD * H], BF16, kind="ExternalInput")
            id_d = nc.dram_tensor("id8_c", [H, H], F32, kind="ExternalInput")
            Es = sp.tile([HD, 8, 8], BF16, name="Es")
            nc.gpsimd.dma_start(out=Es, in_=es_d[:])
            sel = sp.tile([H, HD, H], BF16, name="sel")
            nc.gpsimd.dma_start(out=sel, in_=sel_d[:])
            ident = sp.tile([H, H], F32, name="ident")
            nc.gpsimd.dma_start(out=ident, in_=id_d[:])


            # ---- persistent activation tensors ----
            qT = [pp.tile([HD, N], BF16, name=f"qT{h}") for h in range(H)]
            kT = [pp.tile([HD, N], BF16, name=f"kT{h}") for h in range(H)]
            vext = [pp.tile([128, H, HD + 1], BF16, name=f"v{m}") for m in range(NM)]
            outT = [pp.tile([HD, N], BF16, name=f"oT{h}") for h in range(H)]
            rkt_sb = sp.tile([128, NM, H], F32, name="rkt_sb")  # EXP scales
            rq_bf = sp.tile([H, N], BF16, name="rq_bf")
            sq_nrm = sp.tile([H, N], F32, name="sq_nrm")
            sk_nrm = sp.tile([H, N], F32, name="sk_nrm")
            for m in range(NM):
                nc.vector.memset(vext[m], 1.0)

            # ================= phase 1: projections + norms =================
            with tc.tile_pool(name="sq", bufs=3) as sqp:

                def project_qk(t_list, ss_ps, qkp):
                    """q^T/k^T for heads in t_list (0-7 q, 8-15 k), with
                    split-accumulation (kk 0-2 / 3-5) and stacked norms."""
                    for ti, t_i in enumerate(t_list):
                        col0 = t_i * HD if t_i < 8 else C + (t_i - 8) * HD
                        dst = qT[t_i] if t_i < 8 else kT[t_i - 8]
                        for nc2 in range(2):
                            psA = qkp.tile([HD, 512], F32, name="psA")
                            for kk in range(KC):
                                nc.tensor.matmul(
                                    psA,
                                    lhsT=wqkv_sb[kk][:, col0 : col0 + HD],
                                    rhs=xt_sb[kk][:, nc2 * 512 : (nc2 + 1) * 512],
                                    start=(kk == 0),
                                    stop=(kk == KC - 1),
                                )
                            dslice = dst[:, nc2 * 512 : (nc2 + 1) * 512]
                            nc.vector.tensor_copy(out=dslice, in_=psA)
                            sq = sqp.tile([HD, 512], BF16, name="sq")
                            nc.scalar.activation(out=sq, in_=dslice, func=AF.Square)
                            nc.tensor.matmul(
                                ss_ps[nc2],
                                lhsT=Es[:, t_i % 8, :],
                                rhs=sq,
                                start=(ti == 0),
                                stop=(ti == len(t_list) - 1),
                            )

                with tc.tile_pool(name="p1_ps", bufs=2, space="PSUM") as qkp:
                    # ---- k heads first ----
                    with tc.tile_pool(name="kss_ps", bufs=1, space="PSUM") as kssp:
                        ss_k = [kssp.tile([H, 512], F32, name=f"ssk{i}")
                                for i in range(2)]
                        project_qk(list(range(8, 16)), ss_k, qkp)
                        # k-norm chain: rk = temp / sqrt(ssk)
                        for i in range(2):
                            nc.vector.tensor_copy(
                                out=sk_nrm[:, i * 512 : (i + 1) * 512], in_=ss_k[i]
                            )
                    nc.scalar.activation(out=sk_nrm, in_=sk_nrm, func=AF.Sqrt)
                    nc.vector.reciprocal(out=sk_nrm, in_=sk_nrm)
                    nc.vector.tensor_scalar_mul(
                        out=sk_nrm, in0=sk_nrm, scalar1=temp_col
                    )

                    # ---- q heads + rkt transposes (overlap) ----
                    with tc.tile_pool(name="qss_ps", bufs=1, space="PSUM") as qssp, \
                         tc.tile_pool(name="tr_ps", bufs=1, space="PSUM") as trp:
                        ss_q = [qssp.tile([H, 512], F32, name=f"ssq{i}")
                                for i in range(2)]
                        rkt_ps = trp.tile([128, NM, H], F32, name="rkt_ps")
                        for m in range(NM):
                            nc.tensor.transpose(
                                out=rkt_ps[:, m, :],
                                in_=sk_nrm[:, m * 128 : (m + 1) * 128],
                                identity=ident,
                            )
                        nc.vector.tensor_copy(out=rkt_sb, in_=rkt_ps)
                        project_qk(list(range(0, 8)), ss_q, qkp)
                        # q-norm chain: rq = 1 / sqrt(ssq), bf16 for broadcast
                        for i in range(2):
                            nc.vector.tensor_copy(
                                out=sq_nrm[:, i * 512 : (i + 1) * 512], in_=ss_q[i]
                            )
                    nc.scalar.activation(out=sq_nrm, in_=sq_nrm, func=AF.Sqrt)
                    nc.vector.reciprocal(out=sq_nrm, in_=sq_nrm)
                    nc.vector.tensor_copy(out=rq_bf, in_=sq_nrm)

                # ---- v projections + q normalization (overlap) ----
                with tc.tile_pool(name="v_ps", bufs=2, space="PSUM") as vpp, \
                     tc.tile_pool(name="rqb_ps", bufs=2, space="PSUM") as rqp:

                    def norm_q(h):
                        for nc2 in range(2):
                            rqb = rqp.tile([HD, 512], F32, name="rqb")
                            nc.tensor.matmul(
                                rqb,
                                lhsT=sel[:, :, h],
                                rhs=rq_bf[:, nc2 * 512 : (nc2 + 1) * 512],
                                start=True, stop=True,
                            )
                            sl = qT[h][:, nc2 * 512 : (nc2 + 1) * 512]
                            nc.vector.tensor_mul(out=sl, in0=sl, in1=rqb)

                    # normalize the first heads right away (gates phase 2)
                    norm_q(0)
                    norm_q(1)
                    for nb in range(2):
                        for m in range(NM):
                            ps = vpp.tile([128, 384], F32, name="psv")
                            for kk in range(KC):
                                nc.tensor.matmul(
                                    ps,
                                    lhsT=xt_sb[kk][:, m * 128 : (m + 1) * 128],
                                    rhs=wqkv_sb[kk][
                                        :, 2 * C + nb * 384 : 2 * C + (nb + 1) * 384
                                    ],
                                    start=(kk == 0),
                                    stop=(kk == KC - 1),
                                )
                            nc.vector.tensor_copy(
                                out=vext[m][:, nb * 4 : (nb + 1) * 4, :HD],
                                in_=ps.rearrange("p (hh d) -> p hh d", d=HD),
                            )
                        if nb == 0:
                            for h in (2, 3, 4):
                                norm_q(h)
                    for h in (5, 6, 7):
                        norm_q(h)

            # ================= phase 2: attention per head =================
            with tc.tile_pool(name="pT", bufs=2) as ptp, \
                 tc.tile_pool(name="s_ps", bufs=2, space="PSUM") as spp, \
                 tc.tile_pool(name="o_ps", bufs=2, space="PSUM") as opp, \
                 tc.tile_pool(name="dn", bufs=2) as dnp, \
                 tc.tile_pool(name="dnd", bufs=2, space="DRAM") as ddp:
                for h in range(H):
                    pTs = []
                    for m in range(NM):
                        s_ps = spp.tile([128, N], F32, name="s_ps")
                        for nc2 in range(2):
                            nc.tensor.matmul(
                                s_ps[:, nc2 * 512 : (nc2 + 1) * 512],
                                lhsT=kT[h][:, m * 128 : (m + 1) * 128],
                                rhs=qT[h][:, nc2 * 512 : (nc2 + 1) * 512],
                                start=True, stop=True,
                            )
                        pTm = ptp.tile([128, N], BF16, name=f"pT{m}")
                        nc.scalar.activation(
                            out=pTm, in_=s_ps, func=AF.Exp,
                            scale=rkt_sb[:, m, h : h + 1],
                        )
                        pTs.append(pTm)
                    po = opp.tile([HD + 1, N], F32, name="po")
                    for nc2 in range(2):
                        for m in range(NM):
                            nc.tensor.matmul(
                                po[:, nc2 * 512 : (nc2 + 1) * 512],
                                lhsT=vext[m][:, h, :],
                                rhs=pTs[m][:, nc2 * 512 : (nc2 + 1) * 512],
                                start=(m == 0),
                                stop=(m == NM - 1),
                            )
                    # denominator: DRAM-bounce row 96, broadcast across
                    # partitions, then a single elementwise divide
                    den = dnp.tile([HD + 1, N], F32, name="den")
                    nc.vector.tensor_copy(
                        out=den[HD : HD + 1, :], in_=po[HD : HD + 1, :]
                    )
                    nc.vector.reciprocal(
                        out=den[HD : HD + 1, :], in_=den[HD : HD + 1, :]
                    )
                    d_d = ddp.tile([1, N], F32, name="d_d")
                    nc.sync.dma_start(out=d_d, in_=den[HD : HD + 1, :])
                    rb = dnp.tile([HD, N], F32, name="rb")
                    nc.sync.dma_start(out=rb, in_=_bcast_ap(d_d[:], HD))
                    nc.vector.tensor_mul(out=outT[h], in0=po[:HD, :], in1=rb)

            # ================= phase 3: projection + bias =================
            with tc.tile_pool(name="y_ps", bufs=2, space="PSUM") as ypp, \
                 tc.tile_pool(name="ysb", bufs=2) as ysp:
                for m in range(NM):
                    ym = ysp.tile([128, C], F32, name="ym")
                    for jb in range(2):
                        py = ypp.tile([128, 384], F32, name=f"py{jb}")
                        for h in range(H):
                            nc.tensor.matmul(
                                py,
                                lhsT=outT[h][:, m * 128 : (m + 1) * 128],
                                rhs=wproj_sb[h][:, jb * 384 : (jb + 1) * 384],
                                start=(h == 0),
                                stop=(h == H - 1),
                            )
                        nc.vector.tensor_add(
                            out=ym[:, jb * 384 : (jb + 1) * 384],
                            in0=py,
                            in1=b_bcast[:, jb * 384 : (jb + 1) * 384],
                        )
                    nc.scalar.dma_start(
                        out=y[m * 128 : (m + 1) * 128, :], in_=ym
                    )
    return _split_multi_waits(nc)


_NC = None
LAST_RESULT = None


def kernel(x, w_qkv, temperature, w_proj, b_proj):
    global _NC, LAST_RESULT
    if _NC is None:
        _NC = build()
    xf = np.asarray(x, dtype=np.float32)
    xtb = np.ascontiguousarray(xf.transpose(0, 2, 1)).astype(ml_dtypes.bfloat16)
    wqb = np.asarray(w_qkv, dtype=np.float32).astype(ml_dtypes.bfloat16)
    tf = np.ascontiguousarray(np.asarray(temperature, dtype=np.float32).reshape(H))
    wp = np.asarray(w_proj, dtype=np.float32).astype(ml_dtypes.bfloat16)
    bp = np.ascontiguousarray(np.asarray(b_proj, dtype=np.float32))
    es_c = np.zeros((HD, 8, 8), dtype=ml_dtypes.bfloat16)
    es_c[:, np.arange(8), np.arange(8)] = 1.0
    sel_c = np.zeros((H, HD, H), dtype=ml_dtypes.bfloat16)
    sel_c[np.arange(H), :, np.arange(H)] = 1.0
    id8_c = np.eye(H, dtype=np.float32)
    in_maps = [
        {
            "xt": np.ascontiguousarray(xtb[i]),
            "w_qkv": np.ascontiguousarray(wqb),
            "temperature": tf,
            "w_proj": wp,
            "b_proj": bp,
            "es_c": es_c.reshape(HD, 64),
            "sel_c": sel_c.reshape(H, HD * H),
            "id8_c": id8_c,
        }
        for i in range(B)
    ]
    trace = bool(int(os.environ.get("KERNEL_TRACE", "0")))
    res = run_bass_kernel_spmd(
        _NC, in_maps, core_ids=list(range(B)), trace=trace
    )
    LAST_RESULT = res
    out = np.stack([res.results[i]["y"] for i in range(B)], axis=0)
    return out.astype(np.float32)
